# revision 1
# baseline (speedup 1.0000x reference)
"""Multi-head self-attention on 8 Trainium2 NeuronCores.

Strategy (tensor parallel over heads, per the classic Megatron split):
  - 16 heads / 8 cores -> each core owns 2 heads (a 128-column slice of
    Wq/Wk/Wv and the matching 128-row slice of Wo).
  - x is transposed on the host to xT [D, B*S] and replicated to every
    core; each core projects QT/KT/VT for its heads, runs attention for
    its (batch, head) pairs, and produces a partial output projection
    [B*S, D].
  - Host sums the 8 partials (the Wo row-parallel all-reduce) and adds bo.

Per-core kernel layout notes:
  - All matmuls run in float32r (full PE rate at free-dim >= 256,
    ~1.5e-4 rel rms per matmul vs fp32).
  - Scores are computed transposed, ST[k, q] = KT.T @ QT, two heads
    row-packed into the PE array (contraction is only 64 wide per head).
  - softmax denominator rides the attention matmul: V is augmented with
    a ones column, so AV psum row 64 is sum_k exp(s).
  - exp happens on ACT straight out of PSUM with the 1/8 logit scale.
"""
import sys

sys.path.insert(0, "/opt/trn_rl_repo")

import numpy as np

import concourse.bacc as bacc
import concourse.tile as tile
from concourse import mybir
from concourse.bass_utils import run_bass_kernel_spmd
from concourse.masks import make_identity

AF = mybir.ActivationFunctionType
F32 = mybir.dt.float32
F32R = mybir.dt.float32r

N_CORES = 8
EMBED_DIM = 1024
NUM_HEADS = 16
HEAD_DIM = 64


def build_attention_core(B, S, D, with_qkv_bias=False):
    """One core's program: 2 heads (E=128 projection slice) of MHA.

    B: batch, S: sequence length per batch, D: model dim.
    Inputs: xT [D, B*S], wq/wk/wv [D, 128], wo [128, D], bq/bk/bv [128].
    Output: out [B*S, D] (partial; host sums over cores).
    """
    P = 128          # partitions / d-chunk / k-tile
    E = 128          # per-core projection width (2 heads x 64)
    HD = 64          # head dim
    QC = 512         # q-chunk (matmul moving free dim)
    BS = B * S
    DC = D // P      # number of contraction chunks for projections
    n_sc = BS // QC  # s-chunks for projections
    n_kt = S // P    # k-tiles per batch
    n_qc = S // QC   # q-chunks per batch
    assert BS % QC == 0 and S % P == 0 and S % QC == 0 and D % P == 0
    scale = 1.0 / np.sqrt(np.float32(HD))

    nc = bacc.Bacc("TRN2", target_bir_lowering=False)
    xT = nc.dram_tensor("xT", [D, BS], F32, kind="ExternalInput")
    # wq/wk/wv arrive pre-arranged on host as [P, DC, E] (partition-major)
    # so the load is one fully contiguous DMA instead of 512B descriptors.
    wq = nc.dram_tensor("wq", [P, D // P, E], F32, kind="ExternalInput")
    wk = nc.dram_tensor("wk", [P, D // P, E], F32, kind="ExternalInput")
    wv = nc.dram_tensor("wv", [P, D // P, E], F32, kind="ExternalInput")
    wo = nc.dram_tensor("wo", [E, D], F32, kind="ExternalInput")
    bq = nc.dram_tensor("bq", [E], F32, kind="ExternalInput")
    bk = nc.dram_tensor("bk", [E], F32, kind="ExternalInput")
    bv = nc.dram_tensor("bv", [E], F32, kind="ExternalInput")
    out = nc.dram_tensor("out", [BS, D], F32, kind="ExternalOutput")

    xT_r = xT[:].rearrange("(o p) s -> p o s", p=P)      # [P, DC, BS]
    w_r = {"q": wq[:], "k": wk[:], "v": wv[:]}

    with tile.TileContext(nc) as tc:
        with (
            tc.tile_pool(name="persist", bufs=1) as persist,
            tc.tile_pool(name="stage", bufs=2) as stage,
            tc.tile_pool(name="upool", bufs=5) as upool,
            tc.tile_pool(name="small", bufs=3) as small,
            tc.tile_pool(name="outp", bufs=3) as outp,
            tc.tile_pool(name="psA", bufs=2, space="PSUM") as psA,
            tc.tile_pool(name="psB", bufs=4, space="PSUM") as psB,
        ):
            # ---- x prefetch (first chunk queued before weight DMAs) ------
            _xpre = {}

            def prefetch_x(sc):
                s0 = sc * QC
                xstage = stage.tile([P, DC, QC], F32, tag="xstage")
                xtr = stage.tile([P, DC, QC], F32R, tag="xtr")
                nsplit = min(4, DC)
                dper = DC // nsplit
                for sp in range(nsplit):
                    sl = slice(sp * dper, (sp + 1) * dper)
                    nc.sync.dma_start(xstage[:, sl, :], xT_r[:, sl, s0:s0 + QC])
                    nc.vector.tensor_copy(xtr[:, sl, :], xstage[:, sl, :])
                _xpre[sc] = xtr

            prefetch_x(0)

            # ---- constants & weights -------------------------------------
            ident32 = persist.tile([P, P], F32)
            make_identity(nc, ident32[:])
            ident = persist.tile([P, P], F32R)
            nc.vector.tensor_copy(ident[:], ident32[:])

            ones32 = persist.tile([P, HD], F32)
            nc.gpsimd.memset(ones32[:], 1.0)
            ones_t = persist.tile([P, HD], F32R)
            nc.vector.tensor_copy(ones_t[:], ones32[:])

            bias_t = {}
            if with_qkv_bias:
                for nm, b in (("q", bq), ("k", bk), ("v", bv)):
                    bs32 = persist.tile([P, 1], F32, tag=f"bias32_{nm}")
                    nc.sync.dma_start(bs32[:], b[:].rearrange("(p o) -> p o", o=1))
                    bt = persist.tile([P, 1], F32R, tag=f"bias_{nm}")
                    nc.vector.tensor_copy(bt[:], bs32[:])
                    bias_t[nm] = bt

            w_t = {}
            for nm in ("q", "k", "v"):
                wstage = stage.tile([P, DC, E], F32, tag=f"wst_{nm}", bufs=1)
                nc.sync.dma_start(wstage[:], w_r[nm])
                wt = persist.tile([P, DC, E], F32R, tag=f"w_{nm}")
                nc.vector.tensor_copy(wt[:], wstage[:])
                w_t[nm] = wt
            wo_stage = stage.tile([E, D], F32, tag="wst_o", bufs=1)
            nc.sync.dma_start(wo_stage[:], wo[:])
            wo_t = persist.tile([E, D], F32R)
            nc.vector.tensor_copy(wo_t[:], wo_stage[:])

            # ---- persistent activations ----------------------------------
            QT = persist.tile([P, BS], F32R, tag="QT")     # [e, s]
            KT = persist.tile([P, BS], F32R, tag="KT")     # [e, s]
            # V augmented with ones: per k-chunk [V_h0 | 1 | V_h1 | 1]
            Vaug = persist.tile([P, BS // P, 2 * (HD + 1)], F32R, tag="Vaug")
            nch = BS // P
            assert nch <= HD
            nc.vector.tensor_copy(Vaug[:, :, HD], ones32[:, 0:nch])
            nc.vector.tensor_copy(Vaug[:, :, 2 * HD + 1], ones32[:, 0:nch])

            # ---- phase 1 projections (prefetch_x defined above) ------------
            def emit_proj(sc):
                s0 = sc * QC
                if sc not in _xpre:
                    prefetch_x(sc)
                xtr = _xpre.pop(sc)

                for nm in ("q", "k", "v"):
                    ps = psB.tile([P, QC], F32, tag="B", name=f"ps_{nm}")
                    for o in range(DC):
                        nc.tensor.matmul(
                            ps[:], w_t[nm][:, o, :], xtr[:, o, :],
                            start=(o == 0), stop=(o == DC - 1),
                        )
                    def _bias_add(dst_ap, nm=nm):
                        if with_qkv_bias:
                            nc.vector.tensor_tensor(
                                dst_ap, dst_ap,
                                bias_t[nm][:, 0:1].to_broadcast((P, QC)),
                                mybir.AluOpType.add)
                    if nm == "q":
                        nc.vector.tensor_copy(QT[:, s0:s0 + QC], ps[:])
                        _bias_add(QT[:, s0:s0 + QC])
                    elif nm == "k":
                        nc.vector.tensor_copy(KT[:, s0:s0 + QC], ps[:])
                        _bias_add(KT[:, s0:s0 + QC])
                    else:
                        vt_sb = small.tile([P, QC], F32R, tag="vt")
                        nc.vector.tensor_copy(vt_sb[:], ps[:])
                        _bias_add(vt_sb[:])
                        for ss in range(QC // P):
                            pt = psB.tile([P, P], F32R, tag="B", name="pt")
                            nc.tensor.transpose(
                                pt[:], vt_sb[:, ss * P:(ss + 1) * P], ident[:])
                            ch = sc * (QC // P) + ss
                            nc.vector.tensor_copy(
                                Vaug[:, ch, 0:HD], pt[:, 0:HD])
                            nc.vector.tensor_copy(
                                Vaug[:, ch, HD + 1:2 * HD + 1], pt[:, HD:2 * HD])

            # ---- phase 2: attention + output projection ------------------
            OCW = min(512, D)

            def emit_ktloop(b, qc):
                q0 = b * S + qc * QC
                pa = [psB.tile([HD + 1, QC], F32, tag="B", name=f"pa{h}")
                      for h in range(2)]

                def emit_av(kt, ut):
                    ch = (b * S) // P + kt
                    for h in range(2):
                        nc.tensor.matmul(
                            pa[h][:],
                            Vaug[:, ch, h * (HD + 1):(h + 1) * (HD + 1)],
                            ut[:, h * QC:(h + 1) * QC],
                            start=(kt == 0), stop=(kt == n_kt - 1))

                # AV is emitted one kt behind scores: PE streams the next
                # score pair while ACT's exp of the current tile is in
                # flight, instead of stalling on it.
                prev = None
                for kt in range(n_kt):
                    k0 = b * S + kt * P
                    st = psA.tile([P, 2 * QC], F32, tag="A", name="st")
                    nc.tensor.matmul(
                        st[:, 0:QC],
                        KT[0:HD, k0:k0 + P], QT[0:HD, q0:q0 + QC],
                        tile_position=(0, 0), start=True, stop=True)
                    nc.tensor.matmul(
                        st[:, QC:2 * QC],
                        KT[HD:2 * HD, k0:k0 + P], QT[HD:2 * HD, q0:q0 + QC],
                        tile_position=(64, 0), start=True, stop=True)
                    ut = upool.tile([P, 2 * QC], F32R, tag="U")
                    nc.scalar.activation(ut[:], st[:], AF.Exp, scale=float(scale))
                    if prev is not None:
                        emit_av(*prev)
                    prev = (kt, ut)
                emit_av(*prev)
                return pa

            def emit_tail(b, qc, pa):
                q0 = b * S + qc * QC
                rsb = small.tile([P, 2 * QC], F32, tag="rsb")
                for h in range(2):
                    nc.vector.tensor_copy(
                        rsb[0:1, h * QC:(h + 1) * QC],
                        pa[h][HD:HD + 1, :])
                attnT = small.tile([P, QC], F32R, tag="attnT")
                for h in range(2):
                    prb_sb = small.tile([HD, QC], F32, tag="prb_sb")
                    nc.gpsimd.partition_broadcast(
                        prb_sb[:], rsb[0:1, h * QC:(h + 1) * QC])
                    rinv = small.tile([HD, QC], F32, tag="rinv")
                    nc.vector.reciprocal_approx_fast(rinv[:], prb_sb[:])
                    nc.vector.tensor_tensor(
                        attnT[h * HD:(h + 1) * HD, :],
                        pa[h][0:HD, :], rinv[:],
                        mybir.AluOpType.mult)
                for ss in range(QC // P):
                    for oc in range(D // OCW):
                        po = psB.tile([P, OCW], F32, tag="B", name="po")
                        nc.tensor.matmul(
                            po[:], attnT[:, ss * P:(ss + 1) * P],
                            wo_t[:, oc * OCW:(oc + 1) * OCW],
                            start=True, stop=True)
                        osb = outp.tile([P, OCW], F32, tag="osb")
                        nc.vector.tensor_copy(osb[:], po[:])
                        nc.sync.dma_start(
                            out[q0 + ss * P:q0 + (ss + 1) * P,
                                oc * OCW:(oc + 1) * OCW],
                            osb[:])

            # Interleave: project batch b's s-chunks, then run its attention
            # blocks — the next batch's (DMA-bound) projections overlap the
            # (ACT-bound) attention stream of the current batch.
            per_b = n_sc // B
            for b in range(B):
                if b == 0:
                    for sc in range(per_b):
                        emit_proj(sc)
                for qc in range(n_qc):
                    pa = emit_ktloop(b, qc)
                    # spread the next batch's projections between this
                    # batch's blocks so their DMA+PE hide under the
                    # ACT-bound attention stream
                    if b + 1 < B and qc < per_b:
                        emit_proj((b + 1) * per_b + qc)
                    emit_tail(b, qc, pa)

    nc.compile()
    return nc


_NC_CACHE = {}


def _get_nc(B, S, D, with_qkv_bias):
    key = (B, S, D, with_qkv_bias)
    if key not in _NC_CACHE:
        _NC_CACHE[key] = build_attention_core(B, S, D, with_qkv_bias)
    return _NC_CACHE[key]


def _pack_w(w):
    # [D, 128] -> [128, D//128, 128] partition-major for contiguous DMA
    D = w.shape[0]
    return np.ascontiguousarray(
        w.reshape(D // 128, 128, w.shape[1]).transpose(1, 0, 2))


def run_attention(x, Wq, bq, Wk, bk, Wv, bv, Wo, bo, trace=False):
    B, S, D = x.shape
    with_qkv_bias = bool(np.any(bq) or np.any(bk) or np.any(bv))
    nc = _get_nc(B, S, D, with_qkv_bias)
    xT = np.ascontiguousarray(x.reshape(B * S, D).T)
    in_maps = []
    for c in range(N_CORES):
        sl = slice(c * 128, (c + 1) * 128)
        in_maps.append({
            "xT": xT,
            "wq": _pack_w(Wq[:, sl]),
            "wk": _pack_w(Wk[:, sl]),
            "wv": _pack_w(Wv[:, sl]),
            "wo": np.ascontiguousarray(Wo[sl, :]),
            "bq": np.ascontiguousarray(bq[sl]),
            "bk": np.ascontiguousarray(bk[sl]),
            "bv": np.ascontiguousarray(bv[sl]),
        })
    res = run_bass_kernel_spmd(nc, in_maps, core_ids=list(range(N_CORES)),
                               trace=trace)
    acc = np.asarray(res.results[0]["out"]).astype(np.float32)
    for c in range(1, N_CORES):
        acc = acc + np.asarray(res.results[c]["out"])
    acc = acc + np.asarray(bo, dtype=np.float32)[None, :]
    return acc.reshape(B, S, D), res


def kernel(x, Wq, bq, Wk, bk, Wv, bv, Wo, bo):
    out, _ = run_attention(np.asarray(x), np.asarray(Wq), np.asarray(bq),
                           np.asarray(Wk), np.asarray(bk), np.asarray(Wv),
                           np.asarray(bv), np.asarray(Wo), np.asarray(bo))
    return out



# revision 2
# speedup vs baseline: 1.1537x; 1.1537x over previous
"""Multi-head self-attention on 8 Trainium2 NeuronCores.

Strategy (batch x head-group sharding):
  - 2 batches x 4 head-groups -> each core owns batch b = core//4 and
    heads 4g..4g+3 (g = core%4), i.e. a 256-column slice of Wq/Wk/Wv and
    the matching 256-row slice of Wo, applied to one batch's tokens.
  - All matmul operands are bf16 (cast on the host): full PE rate,
    fast-weight-load enabled, half the SBUF/DMA traffic of fp32.
  - Q/K are projected in [e, s] layout (weights stationary); V is
    projected directly in [s, e] layout (x chunks stationary) so no PE
    transposes are needed. V is stored ones-augmented per head
    ([V_h | 1] 65 columns) so the softmax denominator rides the AV
    matmul as PSUM row 64.
  - Scores are computed transposed, ST[k, q] = K^T Q, two heads
    row-packed into the PE array (64-wide contraction per head).
  - exp on ACT straight out of PSUM with the 1/8 logit scale, bf16 out.
  - Per-head normalization (x 1/denominator) happens on the transposed
    attention matrix right before the output projection; partial outputs
    are written bf16 and the 4 per-batch partials are summed on host
    (the Wo row-parallel all-reduce) with bo added there.

Schedule: pair p=0 attention starts as soon as K/Q/V head-start chunks
are projected; remaining projection work is interleaved into p=0's
kt loops. Output projection of q-chunk qc needs both pairs' attnT, so
it runs during the p=1 phase (when the proj PSUM pool is free).
"""
import sys

sys.path.insert(0, "/opt/trn_rl_repo")

import numpy as np
import ml_dtypes

import concourse.bacc as bacc
import concourse.tile as tile
from concourse import mybir
from concourse.bass_utils import run_bass_kernel_spmd

AF = mybir.ActivationFunctionType
F32 = mybir.dt.float32
BF = mybir.dt.bfloat16
BF_NP = ml_dtypes.bfloat16

N_CORES = 8
D = 1024          # model dim
S = 2048          # tokens per core (one batch)
E = 256           # per-core projection width (4 heads x 64)
HD = 64           # head dim
P = 128           # partitions
QC = 512          # q-chunk
SC = 512          # projection s-chunk (PSUM bank-sized)
DC = D // P       # 8 contraction chunks
N_KT = S // P     # 16 k-tiles
N_QC = S // QC    # 4 q-chunks
N_SC = S // SC    # 4 proj s-chunks
EW = HD + 1       # per-head V width with ones column


def build_attention_core(with_qkv_bias=False):
    scale = 1.0 / np.sqrt(np.float32(HD))

    nc = bacc.Bacc("TRN2", target_bir_lowering=False)
    xT = nc.dram_tensor("xT", [P, DC, S], BF, kind="ExternalInput")
    wq = nc.dram_tensor("wq", [P, DC, E], BF, kind="ExternalInput")
    wk = nc.dram_tensor("wk", [P, DC, E], BF, kind="ExternalInput")
    wv = nc.dram_tensor("wv", [P, DC, E], BF, kind="ExternalInput")
    wo = nc.dram_tensor("wo", [P, 2, D], BF, kind="ExternalInput")
    bq = nc.dram_tensor("bq", [P, 2], F32, kind="ExternalInput")
    bk = nc.dram_tensor("bk", [P, 2], F32, kind="ExternalInput")
    bv = nc.dram_tensor("bv", [1, E], F32, kind="ExternalInput")
    out = nc.dram_tensor("out", [S, D], BF, kind="ExternalOutput")

    with tile.TileContext(nc) as tc:
        with (
            tc.tile_pool(name="persist", bufs=1) as persist,
            tc.tile_pool(name="attp", bufs=6) as attp,
            tc.tile_pool(name="upool", bufs=4) as upool,
            tc.tile_pool(name="small", bufs=3) as small,
            tc.tile_pool(name="outp", bufs=3) as outp,
            tc.tile_pool(name="psS", bufs=2, space="PSUM") as psS,
            tc.tile_pool(name="psP", bufs=2, space="PSUM") as psP,
            tc.tile_pool(name="psQ", bufs=2, space="PSUM") as psQ,
        ):
            # ---- weight + x DMAs (streamed per d-chunk) ------------------
            w_sb = {}
            for nm, t in (("k", wk), ("v", wv), ("q", wq)):
                wt = persist.tile([P, DC, E], BF, tag=f"w_{nm}")
                nc.sync.dma_start(wt[:], t[:])
                w_sb[nm] = wt
            wo_sb = persist.tile([P, 2, D], BF)
            nc.sync.dma_start(wo_sb[:], wo[:])

            x_sb = persist.tile([P, DC, S], BF)
            for o in range(DC):
                nc.sync.dma_start(x_sb[:, o, :], xT[:, o, :])

            bias_qk = {}
            bv_b = None
            if with_qkv_bias:
                for nm, t in (("q", bq), ("k", bk)):
                    bt = persist.tile([P, 2], F32, tag=f"b_{nm}")
                    nc.sync.dma_start(bt[:], t[:])
                    bias_qk[nm] = bt
                bv_sb = persist.tile([1, E], F32, tag="bv_row")
                nc.sync.dma_start(bv_sb[:], bv[:])
                bv_b = persist.tile([P, E], F32, tag="bv_bcast")
                nc.gpsimd.partition_broadcast(bv_b[:], bv_sb[0:1, :])

            # ---- persistent activations ----------------------------------
            KT = persist.tile([P, 2, S], BF, tag="KT")   # [e, slice, s]
            QT = persist.tile([P, 2, S], BF, tag="QT")
            # V ones-augmented: per k-chunk [V_h0|1|V_h1|1|V_h2|1|V_h3|1]
            V_sb = persist.tile([P, N_KT, 4 * EW], BF, tag="V")
            V_r = V_sb[:].rearrange("p c (h u) -> p c h u", u=EW)
            for h in range(4):
                nc.gpsimd.memset(V_r[:, :, h, HD], 1.0)

            # ---- projection emitters -------------------------------------
            def emit_kq(nm, sl, sc):
                """Project K or Q e-slice sl for tokens [sc*SC, (sc+1)*SC)."""
                s0 = sc * SC
                ps = psQ.tile([P, SC], F32, tag="Q", name=f"ps_{nm}")
                for o in range(DC):
                    nc.tensor.matmul(
                        ps[:], w_sb[nm][:, o, sl * P:(sl + 1) * P],
                        x_sb[:, o, s0:s0 + SC],
                        start=(o == 0), stop=(o == DC - 1),
                    )
                dst = (KT if nm == "k" else QT)[:, sl, s0:s0 + SC]
                if with_qkv_bias:
                    nc.vector.tensor_tensor(
                        dst, ps[:],
                        bias_qk[nm][:, sl:sl + 1].to_broadcast((P, SC)),
                        mybir.AluOpType.add)
                else:
                    nc.vector.tensor_copy(dst, ps[:])

            def emit_v(ch):
                """Project V for token tile ch ([ch*128, (ch+1)*128))."""
                s0 = ch * P
                ps = psQ.tile([P, E], F32, tag="Q", name="ps_v")
                for o in range(DC):
                    nc.tensor.matmul(
                        ps[:], x_sb[:, o, s0:s0 + P], w_sb["v"][:, o, :],
                        start=(o == 0), stop=(o == DC - 1),
                    )
                if with_qkv_bias:
                    nc.vector.tensor_tensor(ps[:], ps[:], bv_b[:],
                                            mybir.AluOpType.add)
                # single strided copy into the ones-augmented layout
                dst = V_r[:, ch, :, 0:HD]
                src = ps[:].rearrange("p (h u) -> p h u", u=HD)
                nc.vector.tensor_copy(dst, src)

            # ---- attention -----------------------------------------------
            def emit_block(p, qc, interleave):
                """Scores+exp+AV for head pair p, q-chunk qc.

                interleave: list of callables popped one per kt iteration
                (projection work hidden under the ACT-bound exp stream).
                Returns the pa PSUM tiles (2 heads, ones-row = denom).
                """
                q0 = qc * QC
                pa = [psP.tile([EW, QC], F32, tag="P", name=f"pa{h}")
                      for h in range(2)]

                def emit_av(kt, ut):
                    for h in range(2):
                        nc.tensor.matmul(
                            pa[h][:],
                            V_sb[:, kt, (2 * p + h) * EW:(2 * p + h + 1) * EW],
                            ut[:, h * QC:(h + 1) * QC],
                            start=(kt == 0), stop=(kt == N_KT - 1))

                prev = None
                for kt in range(N_KT):
                    k0 = kt * P
                    st = psS.tile([P, 2 * QC], F32, tag="S", name="st")
                    nc.tensor.matmul(
                        st[:, 0:QC],
                        KT[0:HD, p, k0:k0 + P], QT[0:HD, p, q0:q0 + QC],
                        tile_position=(0, 0), start=True, stop=True)
                    nc.tensor.matmul(
                        st[:, QC:2 * QC],
                        KT[HD:P, p, k0:k0 + P], QT[HD:P, p, q0:q0 + QC],
                        tile_position=(64, 0), start=True, stop=True)
                    ut = upool.tile([P, 2 * QC], BF, tag="U")
                    nc.scalar.activation(ut[:], st[:], AF.Exp,
                                         scale=float(scale))
                    # AV one kt behind: PE streams next scores while ACT
                    # exp of this tile is in flight.
                    if prev is not None:
                        emit_av(*prev)
                    if interleave:
                        interleave.pop(0)()
                    prev = (kt, ut)
                emit_av(*prev)
                return pa

            def emit_tail(p, qc, pa):
                """Normalize pair p's attention -> attnT (bf16, persists)."""
                rsb = small.tile([1, 2 * QC], F32, tag="rsb")
                for h in range(2):
                    nc.vector.tensor_copy(
                        rsb[0:1, h * QC:(h + 1) * QC], pa[h][HD:EW, :])
                rinv1 = small.tile([1, 2 * QC], F32, tag="rinv1")
                nc.vector.reciprocal_approx_fast(rinv1[:], rsb[:])
                rb = small.tile([HD, 2 * QC], F32, tag="rb")
                nc.gpsimd.partition_broadcast(rb[:], rinv1[0:1, :])
                attnT = attp.tile([P, QC], BF, tag=f"attnT_{p}_{qc}")
                for h in range(2):
                    nc.vector.tensor_tensor(
                        attnT[h * HD:(h + 1) * HD, :],
                        pa[h][0:HD, :], rb[:, h * QC:(h + 1) * QC],
                        mybir.AluOpType.mult)
                return attnT

            def emit_outproj(qc, attnT_by_p):
                q0 = qc * QC
                for ss in range(QC // P):
                    for oc in range(2):
                        po = psQ.tile([P, 512], F32, tag="Q", name="po")
                        for p in range(2):
                            nc.tensor.matmul(
                                po[:],
                                attnT_by_p[p][:, ss * P:(ss + 1) * P],
                                wo_sb[:, p, oc * 512:(oc + 1) * 512],
                                start=(p == 0), stop=(p == 1))
                        osb = outp.tile([P, 512], BF, tag="osb")
                        nc.vector.tensor_copy(osb[:], po[:])
                        nc.sync.dma_start(
                            out[q0 + ss * P:q0 + (ss + 1) * P,
                                oc * 512:(oc + 1) * 512],
                            osb[:])

            # ---- schedule ------------------------------------------------
            # Lead-in: K slice0 (all tokens), Q slice0 qc0, first V tiles.
            for sc in range(N_SC):
                emit_kq("k", 0, sc)
            emit_kq("q", 0, 0)
            for ch in range(2):
                emit_v(ch)

            # Remaining projections, interleaved into p=0's kt loops.
            work = [lambda ch=ch: emit_v(ch) for ch in range(2, N_KT)]
            work += [lambda sc=sc: emit_kq("k", 1, sc) for sc in range(N_SC)]
            work += [lambda sc=sc: emit_kq("q", 1, sc) for sc in range(N_SC)]
            nil = lambda: None

            attnT0 = []
            for qc in range(N_QC):
                if qc + 1 < N_QC:
                    emit_kq("q", 0, qc + 1)
                chunk, work = work[:N_KT], work[N_KT:]
                chunk += [nil] * (N_KT - len(chunk))
                pa = emit_block(0, qc, chunk)
                attnT0.append(emit_tail(0, qc, pa))

            for qc in range(N_QC):
                pa = emit_block(1, qc, [nil] * N_KT)
                attnT1 = emit_tail(1, qc, pa)
                emit_outproj(qc, (attnT0[qc], attnT1))

    nc.compile()
    return nc


_NC_CACHE = {}


def _get_nc(with_qkv_bias):
    key = with_qkv_bias
    if key not in _NC_CACHE:
        _NC_CACHE[key] = build_attention_core(with_qkv_bias)
    return _NC_CACHE[key]


def _pack_pdm(a):
    """[D, M] -> [128, D//128, M] partition-major, bf16."""
    Dd, M = a.shape
    return np.ascontiguousarray(
        a.reshape(Dd // P, P, M).transpose(1, 0, 2).astype(BF_NP))


def run_attention(x, Wq, bq, Wk, bk, Wv, bv, Wo, bo, trace=False):
    B, S_, D_ = x.shape
    assert (B, S_, D_) == (2, S, D)
    with_qkv_bias = bool(np.any(bq) or np.any(bk) or np.any(bv))
    nc = _get_nc(with_qkv_bias)
    in_maps = []
    for c in range(N_CORES):
        b, g = divmod(c, N_CORES // 2)
        sl = slice(g * E, (g + 1) * E)
        xTb = np.ascontiguousarray(x[b].T)  # [D, S]
        in_maps.append({
            "xT": _pack_pdm(xTb),
            "wq": _pack_pdm(Wq[:, sl]),
            "wk": _pack_pdm(Wk[:, sl]),
            "wv": _pack_pdm(Wv[:, sl]),
            "wo": np.ascontiguousarray(
                Wo[sl, :].reshape(2, P, D).transpose(1, 0, 2)
                .astype(BF_NP)),
            "bq": np.ascontiguousarray(
                bq[sl].reshape(2, P).T.astype(np.float32)),
            "bk": np.ascontiguousarray(
                bk[sl].reshape(2, P).T.astype(np.float32)),
            "bv": np.ascontiguousarray(
                bv[sl].reshape(1, E).astype(np.float32)),
        })
    res = run_bass_kernel_spmd(nc, in_maps, core_ids=list(range(N_CORES)),
                               trace=trace)
    outs = []
    for b in range(2):
        acc = np.zeros((S, D), dtype=np.float32)
        for g in range(N_CORES // 2):
            acc += np.asarray(res.results[b * 4 + g]["out"]).astype(np.float32)
        outs.append(acc + np.asarray(bo, dtype=np.float32)[None, :])
    return np.stack(outs).reshape(B, S, D), res


def kernel(x, Wq, bq, Wk, bk, Wv, bv, Wo, bo):
    out, _ = run_attention(np.asarray(x), np.asarray(Wq), np.asarray(bq),
                           np.asarray(Wk), np.asarray(bk), np.asarray(Wv),
                           np.asarray(bv), np.asarray(Wo), np.asarray(bo))
    return out


# revision 5
# speedup vs baseline: 1.1714x; 1.0153x over previous
"""Multi-head self-attention on 8 Trainium2 NeuronCores.

Strategy (batch x head-group sharding):
  - 2 batches x 4 head-groups -> each core owns batch b = core//4 and
    heads 4g..4g+3 (g = core%4): a 256-column slice of Wq/Wk/Wv and the
    matching 256-row slice of Wo, applied to one batch's tokens.
  - All matmul operands are bf16 (cast on the host): full PE rate and
    half the SBUF/DMA traffic of fp32.
  - Q/K/V are projected in [e, s] layout (weights stationary, x moving
    at N=1024) -- 8 matmuls + 1 LDWEIGHTS per [128, 1024] output tile.
    V is then flipped to [s, e] via the DMA xbar transpose engine (one
    transpose per (head, half) into a dense tile, then one strided DVE
    copy into the ones-augmented AV layout [V_h | 1]), so the PE never
    transposes anything.
  - Scores are computed transposed, ST[k, q] = K^T Q, two heads
    row-packed into the PE array (64-wide contraction per head).
  - exp on ACT straight out of PSUM with the 1/8 logit scale, bf16 out.
    The softmax denominator rides the AV matmul via the ones column.
  - Per-head normalization happens on the transposed attention matrix
    right before the output projection; partial outputs are written
    bf16 and the 4 per-batch partials are summed on host (the Wo
    row-parallel all-reduce) with bo added there.

Schedule (engines execute in emission order, so placement == schedule):
  lead-in projects K sl0 / V sl0-half0 / Q sl0; attention pair 0 then
  streams ACT-bound while the remaining projection tiles are emitted
  into specific kt slots of its blocks; pair 1 carries the output
  projections of each q-chunk in the following block's kt slots.
PSUM: scores 2x[128,1024] (4 banks) + AV accumulators 2x[65,512]
  (2 banks) + single-buffered proj/outproj [128,1024] (2 banks) = 8.
"""
import sys

sys.path.insert(0, "/opt/trn_rl_repo")

import numpy as np
import ml_dtypes

import concourse.bacc as bacc
import concourse.tile as tile
from concourse import mybir
from concourse.bass_utils import run_bass_kernel_spmd

AF = mybir.ActivationFunctionType
F32 = mybir.dt.float32
BF = mybir.dt.bfloat16
BF_NP = ml_dtypes.bfloat16

N_CORES = 8
D = 1024          # model dim
S = 2048          # tokens per core (one batch)
E = 256           # per-core projection width (4 heads x 64)
HD = 64           # head dim
P = 128           # partitions
QC = 512          # q-chunk
SC = 1024         # projection s-chunk (bf16 moving max)
DC = D // P       # 8
N_KT = S // P     # 16
N_QC = S // QC    # 4
N_SC = S // SC    # 2
EW = HD + 1       # per-head V width with ones column


def build_attention_core(with_qkv_bias=False):
    scale = 1.0 / np.sqrt(np.float32(HD))

    nc = bacc.Bacc("TRN2", target_bir_lowering=False)
    xT = nc.dram_tensor("xT", [P, DC, S], BF, kind="ExternalInput")
    wq = nc.dram_tensor("wq", [P, DC, E], BF, kind="ExternalInput")
    wk = nc.dram_tensor("wk", [P, DC, E], BF, kind="ExternalInput")
    wv = nc.dram_tensor("wv", [P, DC, E], BF, kind="ExternalInput")
    wo = nc.dram_tensor("wo", [P, 2, D], BF, kind="ExternalInput")
    bq = nc.dram_tensor("bq", [P, 2], F32, kind="ExternalInput")
    bk = nc.dram_tensor("bk", [P, 2], F32, kind="ExternalInput")
    bv = nc.dram_tensor("bv", [P, 2], F32, kind="ExternalInput")
    out = nc.dram_tensor("out", [S, D], BF, kind="ExternalOutput")

    with tile.TileContext(nc) as tc:
        with (
            tc.tile_pool(name="persist", bufs=1) as persist,
            tc.tile_pool(name="attp", bufs=6) as attp,
            tc.tile_pool(name="upool", bufs=8) as upool,
            tc.tile_pool(name="vtrp", bufs=2) as vtrp,
            tc.tile_pool(name="small", bufs=3) as small,
            tc.tile_pool(name="outp", bufs=2) as outp,
            tc.tile_pool(name="psS", bufs=2, space="PSUM") as psS,
            tc.tile_pool(name="psP", bufs=2, space="PSUM") as psP,
            tc.tile_pool(name="psQ", bufs=1, space="PSUM") as psQ,
        ):
            # ---- weight + x DMAs (x streamed per (half, d-chunk)) --------
            w_sb = {}
            for nm, t in (("k", wk), ("v", wv), ("q", wq)):
                wt = persist.tile([P, DC, E], BF, tag=f"w_{nm}")
                nc.sync.dma_start(wt[:], t[:])
                w_sb[nm] = wt
            wo_sb = persist.tile([P, 2, D], BF)
            nc.sync.dma_start(wo_sb[:], wo[:])

            x_sb = persist.tile([P, DC, S], BF)
            for sc in range(N_SC):
                for o in range(DC):
                    nc.sync.dma_start(x_sb[:, o, sc * SC:(sc + 1) * SC],
                                      xT[:, o, sc * SC:(sc + 1) * SC])

            bias_t = {}
            if with_qkv_bias:
                for nm, t in (("q", bq), ("k", bk), ("v", bv)):
                    bt = persist.tile([P, 2], F32, tag=f"b_{nm}")
                    nc.sync.dma_start(bt[:], t[:])
                    bias_t[nm] = bt

            # ---- persistent activations ----------------------------------
            KT = persist.tile([P, 2, S], BF, tag="KT")   # [e, slice, s]
            QT = persist.tile([P, 2, S], BF, tag="QT")
            VT = persist.tile([P, 2, S], BF, tag="VT")
            # AV stationary: per k-chunk [V_h0|1|V_h1|1|V_h2|1|V_h3|1]
            V_sb = persist.tile([P, N_KT, 4 * EW], BF, tag="V")
            V_r = V_sb[:].rearrange("p c (h u) -> p c h u", u=EW)
            for h in range(4):
                nc.gpsimd.memset(V_r[:, :, h, HD], 1.0)

            # ---- projection emitters -------------------------------------
            dsts = {"k": KT, "q": QT, "v": VT}

            def emit_proj(nm, sl, sc):
                """Project e-slice sl of K/Q/V for tokens [sc*SC,(sc+1)*SC)."""
                s0 = sc * SC
                ps = psQ.tile([P, SC], F32, tag="Q", name=f"ps_{nm}")
                for o in range(DC):
                    for hh in range(SC // 512):
                        nc.tensor.matmul(
                            ps[:, hh * 512:(hh + 1) * 512],
                            w_sb[nm][:, o, sl * P:(sl + 1) * P],
                            x_sb[:, o, s0 + hh * 512:s0 + (hh + 1) * 512],
                            start=(o == 0), stop=(o == DC - 1),
                        )
                dst = dsts[nm][:, sl, s0:s0 + SC]
                if with_qkv_bias:
                    nc.vector.tensor_tensor(
                        dst, ps[:],
                        bias_t[nm][:, sl:sl + 1].to_broadcast((P, SC)),
                        mybir.AluOpType.add)
                else:
                    nc.vector.tensor_copy(dst, ps[:])

            def emit_vflip(h, sc):
                """Transpose head h's V tokens [sc*SC,(sc+1)*SC) into V_sb."""
                sl, h2 = divmod(h, 2)
                vtr = vtrp.tile([P, SC // P, HD], BF, tag="vtr")
                nc.sync.dma_start_transpose(
                    vtr[:],
                    VT[h2 * HD:(h2 + 1) * HD, sl, sc * SC:(sc + 1) * SC])
                c0 = sc * (SC // P)
                nc.vector.tensor_copy(
                    V_r[:, c0:c0 + SC // P, h, 0:HD], vtr[:])

            # ---- attention -----------------------------------------------
            def emit_block(p, qc, sched):
                """Scores+exp+AV for head pair p, q-chunk qc.

                sched: {kt: [callables]} -- projection/outproj work emitted
                into that kt slot (hidden under the ACT-bound exp stream).
                """
                q0 = qc * QC
                pa = [psP.tile([EW, QC], F32, tag="P", name=f"pa{h}")
                      for h in range(2)]

                def emit_av(kt, ut):
                    for h in range(2):
                        nc.tensor.matmul(
                            pa[h][:],
                            V_sb[:, kt, (2 * p + h) * EW:(2 * p + h + 1) * EW],
                            ut[:, h * QC:(h + 1) * QC],
                            start=(kt == 0), stop=(kt == N_KT - 1))

                prev = None
                for kt in range(N_KT):
                    k0 = kt * P
                    st = psS.tile([P, 2 * QC], F32, tag="S", name="st")
                    nc.tensor.matmul(
                        st[:, 0:QC],
                        KT[0:HD, p, k0:k0 + P], QT[0:HD, p, q0:q0 + QC],
                        tile_position=(0, 0), start=True, stop=True)
                    nc.tensor.matmul(
                        st[:, QC:2 * QC],
                        KT[HD:P, p, k0:k0 + P], QT[HD:P, p, q0:q0 + QC],
                        tile_position=(64, 0), start=True, stop=True)
                    ut = upool.tile([P, 2 * QC], BF, tag="U")
                    nc.scalar.activation(ut[:], st[:], AF.Exp,
                                         scale=float(scale))
                    if prev is not None:
                        emit_av(*prev)
                    for fn in sched.get(kt, ()):
                        fn()
                    prev = (kt, ut)
                emit_av(*prev)
                return pa

            def emit_tail(p, qc, pa):
                """Normalize pair p's attention -> attnT (bf16, persists)."""
                rsb = small.tile([1, 2 * QC], F32, tag="rsb")
                for h in range(2):
                    nc.vector.tensor_copy(
                        rsb[0:1, h * QC:(h + 1) * QC], pa[h][HD:EW, :])
                rinv1 = small.tile([1, 2 * QC], F32, tag="rinv1")
                nc.vector.reciprocal_approx_fast(rinv1[:], rsb[:])
                rb = small.tile([HD, 2 * QC], F32, tag="rb")
                nc.gpsimd.partition_broadcast(rb[:], rinv1[0:1, :])
                attnT = attp.tile([P, QC], BF, tag=f"attnT_{p}_{qc}")
                for h in range(2):
                    nc.vector.tensor_tensor(
                        attnT[h * HD:(h + 1) * HD, :],
                        pa[h][0:HD, :], rb[:, h * QC:(h + 1) * QC],
                        mybir.AluOpType.mult)
                return attnT

            def emit_outproj_ss(qc, ss, attnT_by_p):
                q0 = qc * QC
                po = psQ.tile([P, D], F32, tag="Q", name="po")
                for p in range(2):
                    for oc in range(D // 512):
                        nc.tensor.matmul(
                            po[:, oc * 512:(oc + 1) * 512],
                            attnT_by_p[p][:, ss * P:(ss + 1) * P],
                            wo_sb[:, p, oc * 512:(oc + 1) * 512],
                            start=(p == 0), stop=(p == 1))
                osb = outp.tile([P, D], BF, tag="osb")
                nc.vector.tensor_copy(osb[:], po[:])
                nc.sync.dma_start(
                    out[q0 + ss * P:q0 + (ss + 1) * P, :], osb[:])

            # ---- schedule ------------------------------------------------
            # Lead-in: K sl0 half0 -> V heads 0/1 half0 -> Q sl0 half0
            # -> K sl0 half1 (kt8+ of qc0/qc1).
            emit_proj("k", 0, 0)
            emit_proj("v", 0, 0)
            emit_vflip(0, 0)
            emit_vflip(1, 0)
            emit_proj("q", 0, 0)
            emit_proj("k", 0, 1)

            J = lambda nm, sl, sc: (lambda: emit_proj(nm, sl, sc))
            F = lambda h, sc: (lambda: emit_vflip(h, sc))
            p0_sched = [
                {0: [J("v", 0, 1)], 5: [F(0, 1)], 7: [F(1, 1)],
                 9: [J("k", 1, 0)], 13: [J("q", 0, 1)]},          # qc0
                {1: [J("k", 1, 1)], 5: [J("q", 1, 0)],
                 9: [J("v", 1, 0)], 14: [F(2, 0)]},               # qc1
                {0: [F(3, 0)], 2: [J("v", 1, 1)], 7: [F(2, 1)],
                 9: [F(3, 1)], 12: [J("q", 1, 1)]},               # qc2
                {},                                               # qc3
            ]

            attnT0 = []
            for qc in range(N_QC):
                pa = emit_block(0, qc, p0_sched[qc])
                attnT0.append(emit_tail(0, qc, pa))

            attnT1_prev = None
            for qc in range(N_QC):
                sched = {}
                if attnT1_prev is not None:
                    pair = (attnT0[qc - 1], attnT1_prev)
                    sched = {2 + 4 * ss: [
                        (lambda ss=ss, pair=pair, q=qc - 1:
                         emit_outproj_ss(q, ss, pair))]
                        for ss in range(QC // P)}
                pa = emit_block(1, qc, sched)
                attnT1_prev = emit_tail(1, qc, pa)
            for ss in range(QC // P):
                emit_outproj_ss(N_QC - 1, ss, (attnT0[-1], attnT1_prev))

    nc.compile()
    return nc


_NC_CACHE = {}


def _get_nc(with_qkv_bias):
    key = with_qkv_bias
    if key not in _NC_CACHE:
        _NC_CACHE[key] = build_attention_core(with_qkv_bias)
    return _NC_CACHE[key]


def _pack_pdm(a):
    """[D, M] -> [128, D//128, M] partition-major, bf16."""
    Dd, M = a.shape
    return np.ascontiguousarray(
        a.reshape(Dd // P, P, M).transpose(1, 0, 2).astype(BF_NP))


def run_attention(x, Wq, bq, Wk, bk, Wv, bv, Wo, bo, trace=False):
    B, S_, D_ = x.shape
    assert (B, S_, D_) == (2, S, D)
    with_qkv_bias = bool(np.any(bq) or np.any(bk) or np.any(bv))
    nc = _get_nc(with_qkv_bias)
    in_maps = []
    for c in range(N_CORES):
        b, g = divmod(c, N_CORES // 2)
        sl = slice(g * E, (g + 1) * E)
        xTb = np.ascontiguousarray(x[b].T)  # [D, S]
        in_maps.append({
            "xT": _pack_pdm(xTb),
            "wq": _pack_pdm(Wq[:, sl]),
            "wk": _pack_pdm(Wk[:, sl]),
            "wv": _pack_pdm(Wv[:, sl]),
            "wo": np.ascontiguousarray(
                Wo[sl, :].reshape(2, P, D).transpose(1, 0, 2)
                .astype(BF_NP)),
            "bq": np.ascontiguousarray(
                bq[sl].reshape(2, P).T.astype(np.float32)),
            "bk": np.ascontiguousarray(
                bk[sl].reshape(2, P).T.astype(np.float32)),
            "bv": np.ascontiguousarray(
                bv[sl].reshape(2, P).T.astype(np.float32)),
        })
    res = run_bass_kernel_spmd(nc, in_maps, core_ids=list(range(N_CORES)),
                               trace=trace)
    outs = []
    for b in range(2):
        acc = np.zeros((S, D), dtype=np.float32)
        for g in range(N_CORES // 2):
            acc += np.asarray(res.results[b * 4 + g]["out"]).astype(np.float32)
        outs.append(acc + np.asarray(bo, dtype=np.float32)[None, :])
    return np.stack(outs).reshape(B, S, D), res


def kernel(x, Wq, bq, Wk, bk, Wv, bv, Wo, bo):
    out, _ = run_attention(np.asarray(x), np.asarray(Wq), np.asarray(bq),
                           np.asarray(Wk), np.asarray(bk), np.asarray(Wv),
                           np.asarray(bv), np.asarray(Wo), np.asarray(bo))
    return out


# revision 10
# speedup vs baseline: 1.2453x; 1.0631x over previous
"""Multi-head self-attention on 8 Trainium2 NeuronCores.

Strategy (batch x head-group sharding):
  - 2 batches x 4 head-groups -> each core owns batch b = core//4 and
    heads 4g..4g+3 (g = core%4): a 256-column slice of Wq/Wk/Wv and the
    matching 256-row slice of Wo, applied to one batch's tokens.
  - All matmul operands are bf16 (cast on the host): full PE rate and
    half the SBUF/DMA traffic of fp32.
  - Q/K/V are projected in [e, s] layout (weights stationary); V is
    then flipped to [s, e] via the DMA xbar transpose engine (one
    transpose per (head, half) into a dense tile, then strided DVE
    copies into the ones-augmented AV layout [V_h | 1]), so the PE
    never transposes anything.
  - Scores are computed transposed, ST[k, q] = K^T Q, two heads
    row-packed into the PE array (64-wide contraction per head).
  - softmax exp: most k-tiles on ACT (table exp, bf16 out); 3 of every
    16 on the DVE via the Schraudolph bit trick (affine fp32->int32 in
    one tensor_scalar, then an f32r rounding copy), which keeps the
    ACT exp stream -- the critical path -- 19% shorter. The softmax
    denominator rides the AV matmul via the ones column.
  - Per-head normalization happens on the transposed attention matrix
    right before the output projection; partial outputs are written
    bf16 and the 4 per-batch partials are summed on host (the Wo
    row-parallel all-reduce) with bo added there.

Schedule (engines execute in emission order, so placement == schedule):
  x is DMAd in 4 big chunks split across the two HWDGE queues (sync +
  activation) with wk first; the lead-in projects only K sl0 / Q sl0
  for the first 1024 tokens, then attention pair 0 streams ACT-bound
  while every other projection tile is emitted (in half-tile items)
  into specific kt slots of its blocks. AV matmuls trail the exp
  stream by 5-8 k-tiles (catching up over the last slots) so a block's
  first AV never waits on the previous block's tail chain (in-order
  engines: a blocked instruction stalls everything behind it). Pair 1
  carries the output projections of each q-chunk in the next block.
PSUM: scores 2x[128,1024] (4 banks) + AV accumulators 2x[65,512]
  (2 banks) + single-buffered proj/outproj [128,1024] (2 banks) = 8.
"""
import sys

sys.path.insert(0, "/opt/trn_rl_repo")

import numpy as np
import ml_dtypes

import concourse.bacc as bacc
import concourse.tile as tile
from concourse import mybir
from concourse.bass_utils import run_bass_kernel_spmd

AF = mybir.ActivationFunctionType
F32 = mybir.dt.float32
F32R = mybir.dt.float32r
I32 = mybir.dt.int32
BF = mybir.dt.bfloat16
BF_NP = ml_dtypes.bfloat16

N_CORES = 8
D = 1024          # model dim
S = 2048          # tokens per core (one batch)
E = 256           # per-core projection width (4 heads x 64)
HD = 64           # head dim
P = 128           # partitions
QC = 512          # q-chunk
SC = 1024         # projection s-chunk
DC = D // P       # 8
N_KT = S // P     # 16
N_QC = S // QC    # 4
N_SC = S // SC    # 2
EW = HD + 1       # per-head V width with ones column

SCHR_KT = (4, 9, 14)     # k-tiles whose exp runs on DVE (Schraudolph)
SCHR_A = float(2**23 / np.log(2.0)) / 8.0          # folds the 1/8 scale
SCHR_B = float(127 * 2**23 - 0.043677448 * 2**23 + 0.5)


def build_attention_core(with_qkv_bias=False):
    scale = 1.0 / np.sqrt(np.float32(HD))

    nc = bacc.Bacc("TRN2", target_bir_lowering=False)
    xT = nc.dram_tensor("xT", [P, DC, S], BF, kind="ExternalInput")
    wq = nc.dram_tensor("wq", [P, DC, E], BF, kind="ExternalInput")
    wk = nc.dram_tensor("wk", [P, DC, E], BF, kind="ExternalInput")
    wv = nc.dram_tensor("wv", [P, DC, E], BF, kind="ExternalInput")
    wo = nc.dram_tensor("wo", [P, 2, D], BF, kind="ExternalInput")
    bq = nc.dram_tensor("bq", [P, 2], F32, kind="ExternalInput")
    bk = nc.dram_tensor("bk", [P, 2], F32, kind="ExternalInput")
    bv = nc.dram_tensor("bv", [P, 2], F32, kind="ExternalInput")
    out = nc.dram_tensor("out", [S, D], BF, kind="ExternalOutput")

    with tile.TileContext(nc) as tc:
        with (
            tc.tile_pool(name="persist", bufs=1) as persist,
            tc.tile_pool(name="attp", bufs=6) as attp,
            tc.tile_pool(name="upool", bufs=8) as upool,
            tc.tile_pool(name="u32p", bufs=1) as u32p,
            tc.tile_pool(name="urp", bufs=3) as urp,
            tc.tile_pool(name="vtrp", bufs=2) as vtrp,
            tc.tile_pool(name="small", bufs=2) as small,
            tc.tile_pool(name="outp", bufs=2) as outp,
            tc.tile_pool(name="psS", bufs=2, space="PSUM") as psS,
            tc.tile_pool(name="psP", bufs=2, space="PSUM") as psP,
            tc.tile_pool(name="psQ", bufs=1, space="PSUM") as psQ,
        ):
            # ---- DMAs: wk first, x in 4 big chunks on both queues --------
            w_sb = {}
            for nm in ("k", "v", "q"):
                w_sb[nm] = persist.tile([P, DC, E], BF, tag=f"w_{nm}",
                                        name=f"w_{nm}")
            wo_sb = persist.tile([P, 2, D], BF)
            x_sb = persist.tile([P, DC, S], BF)

            nc.sync.dma_start(w_sb["k"][:], wk[:])
            H = DC // 2
            for sc in range(N_SC):
                nc.sync.dma_start(x_sb[:, 0:H, sc * SC:(sc + 1) * SC],
                                  xT[:, 0:H, sc * SC:(sc + 1) * SC])
                nc.scalar.dma_start(x_sb[:, H:DC, sc * SC:(sc + 1) * SC],
                                    xT[:, H:DC, sc * SC:(sc + 1) * SC])
            nc.scalar.dma_start(w_sb["q"][:], wq[:])
            nc.scalar.dma_start(w_sb["v"][:], wv[:])
            nc.scalar.dma_start(wo_sb[:], wo[:])

            bias_t = {}
            if with_qkv_bias:
                for nm, t in (("q", bq), ("k", bk), ("v", bv)):
                    bt = persist.tile([P, 2], F32, tag=f"b_{nm}")
                    nc.sync.dma_start(bt[:], t[:])
                    bias_t[nm] = bt

            # ---- persistent activations ----------------------------------
            KT = persist.tile([P, 2, S], BF, tag="KT")   # [e, slice, s]
            QT = persist.tile([P, 2, S], BF, tag="QT")
            VT = persist.tile([P, 2, S], BF, tag="VT")
            # AV stationary: per k-chunk [V_h0|1|V_h1|1|V_h2|1|V_h3|1]
            V_sb = persist.tile([P, N_KT, 4 * EW], BF, tag="V")
            V_r = V_sb[:].rearrange("p c (h u) -> p c h u", u=EW)
            V32 = persist.tile([P, N_KT, 4 * EW], F32R, tag="V32")
            V32_r = V32[:].rearrange("p c (h u) -> p c h u", u=EW)
            V32f_r = V32[:].bitcast(F32).rearrange("p c (h u) -> p c h u", u=EW)
            for h in range(4):
                nc.gpsimd.memset(V_r[:, :, h, HD], 1.0)
                nc.gpsimd.memset(V32f_r[:, :, h, HD], 1.0)

            # ---- projection emitters (two-half items) --------------------
            dsts = {"k": KT, "q": QT, "v": VT}

            def proj_h1(nm, sl, sc):
                s0 = sc * SC
                ps = psQ.tile([P, SC], F32, tag="Q", name=f"ps_{nm}")
                for o in range(DC // 2):
                    for hh in range(SC // 512):
                        nc.tensor.matmul(
                            ps[:, hh * 512:(hh + 1) * 512],
                            w_sb[nm][:, o, sl * P:(sl + 1) * P],
                            x_sb[:, o, s0 + hh * 512:s0 + (hh + 1) * 512],
                            start=(o == 0), stop=False,
                        )
                return ps

            def proj_h2(nm, sl, sc, ps):
                s0 = sc * SC
                for o in range(DC // 2, DC):
                    for hh in range(SC // 512):
                        nc.tensor.matmul(
                            ps[:, hh * 512:(hh + 1) * 512],
                            w_sb[nm][:, o, sl * P:(sl + 1) * P],
                            x_sb[:, o, s0 + hh * 512:s0 + (hh + 1) * 512],
                            start=False, stop=(o == DC - 1),
                        )
                dst = dsts[nm][:, sl, s0:s0 + SC]
                if with_qkv_bias:
                    nc.vector.tensor_tensor(
                        dst, ps[:],
                        bias_t[nm][:, sl:sl + 1].to_broadcast((P, SC)),
                        mybir.AluOpType.add)
                else:
                    nc.vector.tensor_copy(dst, ps[:])

            def emit_proj(nm, sl, sc):
                proj_h2(nm, sl, sc, proj_h1(nm, sl, sc))

            def emit_vflip(h, sc):
                """Transpose head h's V tokens [sc*SC,(sc+1)*SC) into V_sb."""
                sl, h2 = divmod(h, 2)
                vtr = vtrp.tile([P, SC // P, HD], BF, tag="vtr")
                nc.sync.dma_start_transpose(
                    vtr[:],
                    VT[h2 * HD:(h2 + 1) * HD, sl, sc * SC:(sc + 1) * SC])
                c0 = sc * (SC // P)
                nc.vector.tensor_copy(
                    V_r[:, c0:c0 + SC // P, h, 0:HD], vtr[:])
                nc.vector.tensor_copy(
                    V32_r[:, c0:c0 + SC // P, h, 0:HD], vtr[:])

            # ---- attention -----------------------------------------------
            def emit_block(p, qc, sched, lag=5):
                """Scores+exp+AV for head pair p, q-chunk qc.

                sched: {kt: [callables]} -- projection/outproj work emitted
                into that kt slot. AV trails exp by `lag` k-tiles, catching
                up over the last `lag` slots.
                """
                q0 = qc * QC
                pa = [psP.tile([EW, QC], F32, tag="P", name=f"pa{h}")
                      for h in range(2)]

                def emit_av(kt, ut, f32r):
                    vsrc = V32 if f32r else V_sb
                    for h in range(2):
                        nc.tensor.matmul(
                            pa[h][:],
                            vsrc[:, kt, (2 * p + h) * EW:(2 * p + h + 1) * EW],
                            ut[:, h * QC:(h + 1) * QC],
                            start=(kt == 0), stop=(kt == N_KT - 1))

                pend = []
                for kt in range(N_KT):
                    k0 = kt * P
                    st = psS.tile([P, 2 * QC], F32, tag="S", name="st")
                    nc.tensor.matmul(
                        st[:, 0:QC],
                        KT[0:HD, p, k0:k0 + P], QT[0:HD, p, q0:q0 + QC],
                        tile_position=(0, 0), start=True, stop=True)
                    nc.tensor.matmul(
                        st[:, QC:2 * QC],
                        KT[HD:P, p, k0:k0 + P], QT[HD:P, p, q0:q0 + QC],
                        tile_position=(64, 0), start=True, stop=True)
                    if kt in SCHR_KT:
                        u32 = u32p.tile([P, 2 * QC], I32, tag="U32")
                        nc.vector.tensor_scalar(
                            u32[:], st[:], SCHR_A, SCHR_B,
                            mybir.AluOpType.mult, mybir.AluOpType.add)
                        ur = urp.tile([P, 2 * QC], F32R, tag="UR")
                        nc.vector.tensor_copy(ur[:], u32[:].bitcast(F32))
                        pend.append((kt, ur, True))
                    else:
                        ut = upool.tile([P, 2 * QC], BF, tag="U")
                        nc.scalar.activation(ut[:], st[:], AF.Exp,
                                             scale=float(scale))
                        pend.append((kt, ut, False))
                    hi = kt - lag + max(0, kt - (N_KT - 1 - lag))
                    while pend and pend[0][0] <= hi:
                        emit_av(*pend.pop(0))
                    for fn in sched.get(kt, ()):
                        fn()
                for item in pend:
                    emit_av(*item)
                return pa

            def emit_tail(p, qc, pa):
                """Normalize pair p's attention -> attnT (bf16, persists)."""
                rsb = small.tile([1, 2 * QC], F32, tag="rsb")
                for h in range(2):
                    nc.vector.tensor_copy(
                        rsb[0:1, h * QC:(h + 1) * QC], pa[h][HD:EW, :])
                rinv1 = small.tile([1, 2 * QC], F32, tag="rinv1")
                nc.vector.reciprocal_approx_fast(rinv1[:], rsb[:])
                rb = small.tile([HD, 2 * QC], F32, tag="rb")
                nc.gpsimd.partition_broadcast(rb[:], rinv1[0:1, :])
                attnT = attp.tile([P, QC], BF, tag=f"attnT_{p}_{qc}")
                for h in range(2):
                    nc.vector.tensor_tensor(
                        attnT[h * HD:(h + 1) * HD, :],
                        pa[h][0:HD, :], rb[:, h * QC:(h + 1) * QC],
                        mybir.AluOpType.mult)
                return attnT

            def emit_outproj_ss(qc, ss, attnT_by_p):
                q0 = qc * QC
                po = psQ.tile([P, D], F32, tag="Q", name="po")
                for p in range(2):
                    for oc in range(D // 512):
                        nc.tensor.matmul(
                            po[:, oc * 512:(oc + 1) * 512],
                            attnT_by_p[p][:, ss * P:(ss + 1) * P],
                            wo_sb[:, p, oc * 512:(oc + 1) * 512],
                            start=(p == 0), stop=(p == 1))
                osb = outp.tile([P, D], BF, tag="osb")
                nc.vector.tensor_copy(osb[:], po[:])
                nc.sync.dma_start(
                    out[q0 + ss * P:q0 + (ss + 1) * P, :], osb[:])

            # ---- schedule ------------------------------------------------
            emit_proj("k", 0, 0)
            emit_proj("q", 0, 0)

            ctx = {}

            def H1(nm, sl, sc):
                return lambda: ctx.__setitem__(
                    (nm, sl, sc), proj_h1(nm, sl, sc))

            def H2(nm, sl, sc):
                return lambda: proj_h2(nm, sl, sc, ctx.pop((nm, sl, sc)))

            F = lambda h, sc: (lambda: emit_vflip(h, sc))
            p0_sched = [
                {0: [H1("k", 0, 1)], 1: [H2("k", 0, 1)],
                 2: [H1("v", 0, 0)], 3: [H2("v", 0, 0)],
                 4: [F(0, 0)], 5: [F(1, 0)],
                 6: [H1("v", 0, 1)], 7: [H2("v", 0, 1)],
                 8: [F(0, 1)], 9: [F(1, 1)]},                   # qc0
                {0: [H1("q", 0, 1)], 2: [H2("q", 0, 1)],
                 4: [H1("k", 1, 0)], 6: [H2("k", 1, 0)],
                 8: [H1("v", 1, 0)], 10: [H2("v", 1, 0)],
                 12: [F(2, 0)], 13: [F(3, 0)]},                 # qc1
                {0: [H1("q", 1, 0)], 2: [H2("q", 1, 0)]},       # qc2
                {},                                             # qc3
            ]
            p1_extra = [
                {0: [H1("k", 1, 1)], 2: [H2("k", 1, 1)],
                 4: [H1("v", 1, 1)], 6: [H2("v", 1, 1)],
                 8: [F(2, 1)], 10: [F(3, 1)]},                  # p1 qc0
                {0: [H1("q", 1, 1)], 2: [H2("q", 1, 1)]},       # p1 qc1
                {}, {},
            ]

            attnT0 = []
            for qc in range(N_QC):
                pa = emit_block(0, qc, p0_sched[qc], lag=(8 if qc == 0 else 5))
                attnT0.append(emit_tail(0, qc, pa))

            attnT1_prev = None
            for qc in range(N_QC):
                sched = dict(p1_extra[qc])
                if attnT1_prev is not None:
                    pair = (attnT0[qc - 1], attnT1_prev)
                    for ss in range(QC // P):
                        sched.setdefault(4 + 3 * ss, []).append(
                            (lambda ss=ss, pair=pair, q=qc - 1:
                             emit_outproj_ss(q, ss, pair)))
                pa = emit_block(1, qc, sched, lag=5)
                attnT1_prev = emit_tail(1, qc, pa)
            for ss in range(QC // P):
                emit_outproj_ss(N_QC - 1, ss, (attnT0[-1], attnT1_prev))

    nc.compile()
    return nc


_NC_CACHE = {}


def _get_nc(with_qkv_bias):
    key = with_qkv_bias
    if key not in _NC_CACHE:
        _NC_CACHE[key] = build_attention_core(with_qkv_bias)
    return _NC_CACHE[key]


def _pack_pdm(a):
    """[D, M] -> [128, D//128, M] partition-major, bf16."""
    Dd, M = a.shape
    return np.ascontiguousarray(
        a.reshape(Dd // P, P, M).transpose(1, 0, 2).astype(BF_NP))


def run_attention(x, Wq, bq, Wk, bk, Wv, bv, Wo, bo, trace=False):
    B, S_, D_ = x.shape
    assert (B, S_, D_) == (2, S, D)
    with_qkv_bias = bool(np.any(bq) or np.any(bk) or np.any(bv))
    nc = _get_nc(with_qkv_bias)
    in_maps = []
    for c in range(N_CORES):
        b, g = divmod(c, N_CORES // 2)
        sl = slice(g * E, (g + 1) * E)
        xTb = np.ascontiguousarray(x[b].T)  # [D, S]
        in_maps.append({
            "xT": _pack_pdm(xTb),
            "wq": _pack_pdm(Wq[:, sl]),
            "wk": _pack_pdm(Wk[:, sl]),
            "wv": _pack_pdm(Wv[:, sl]),
            "wo": np.ascontiguousarray(
                Wo[sl, :].reshape(2, P, D).transpose(1, 0, 2)
                .astype(BF_NP)),
            "bq": np.ascontiguousarray(
                bq[sl].reshape(2, P).T.astype(np.float32)),
            "bk": np.ascontiguousarray(
                bk[sl].reshape(2, P).T.astype(np.float32)),
            "bv": np.ascontiguousarray(
                bv[sl].reshape(2, P).T.astype(np.float32)),
        })
    res = run_bass_kernel_spmd(nc, in_maps, core_ids=list(range(N_CORES)),
                               trace=trace)
    outs = []
    for b in range(2):
        acc = np.zeros((S, D), dtype=np.float32)
        for g in range(N_CORES // 2):
            acc += np.asarray(res.results[b * 4 + g]["out"]).astype(np.float32)
        outs.append(acc + np.asarray(bo, dtype=np.float32)[None, :])
    return np.stack(outs).reshape(B, S, D), res


def kernel(x, Wq, bq, Wk, bk, Wv, bv, Wo, bo):
    out, _ = run_attention(np.asarray(x), np.asarray(Wq), np.asarray(bq),
                           np.asarray(Wk), np.asarray(bk), np.asarray(Wv),
                           np.asarray(bv), np.asarray(Wo), np.asarray(bo))
    return out
